# revision 9
# baseline (speedup 1.0000x reference)
"""GAT (graph attention) kernel for Trainium2, 8 NeuronCores, v2.

Math identical to the baseline kernel: per head h,
    u[j,i] = mask[j,i] * max(P'_i * QT_j, T_j)
with P'_i = e^{0.8 src_i}, QT_j = e^{tgt_j}, T_j = e^{0.2 tgt_j}; the row
factor e^{0.2 src_i} cancels in the softmax.  Denominator comes from a
ones-column in V.

v2 changes vs baseline:
  * 2D sharding: SI i-shards x SH head-shards (SI*SH = 8 cores).  Each core
    owns RI = N/SI destination rows and HC = H/SH heads; the host adds the
    SH partial outputs per i-slice (mean over heads is linear).
  * Transposed attention matmuls: out[i,65] = ut[j,i]^T-contract V65[j,65].
    Output free size 65 instead of R -> ~2x less PE time.
  * Mask multiplies split between DVE and GpSimd (Pool) so the two engines
    work in parallel; everything else identical math.
"""

import numpy as np

from contextlib import ExitStack

import concourse.bass as bass
import concourse.bacc as bacc
import concourse.mybir as mybir
import concourse.tile as tile
from concourse.bass_utils import run_bass_kernel_spmd
from concourse.masks import make_identity

N, E, F_IN, H, D = 2048, 4096, 256, 8, 64
NCORES = 8
SI = 4                   # i-shards (destination-row shards)
SH = NCORES // SI        # head shards
RI = N // SI             # destination rows per core
IC = RI // 128           # i chunks of 128
HC = H // SH             # heads per core
NCH = N // 128           # 16 node j-chunks
ECH = E // 128           # 32 edge j-chunks
U = 2 * HC               # (head, part) broadcast rows
G = 4                    # j-chunks per fused mask-multiply group
F16 = mybir.dt.float16
F32 = mybir.dt.float32

# chunks per mask-multiply group handed to GpSimd, per group index within a
# (part, head) sweep: front-loaded so the Pool engine's backlog drains before
# the head's last (PSUM-stopping) matmuls and never runs past DVE at the tail.
GPC_N = [1, 1, 1, 1]
GPC_E = [2, 2, 2, 1, 1, 1, 1, 0]

_PROGRAM = None


def _build_program() -> bass.Bass:
    nc = bacc.Bacc("TRN2", target_bir_lowering=False, debug=False)

    maskN_d = nc.dram_tensor("maskN", [N, RI], F16, kind="ExternalInput")
    maskE_d = nc.dram_tensor("maskE", [E, RI], F16, kind="ExternalInput")
    nodesT_d = nc.dram_tensor("nodesT", [F_IN, N], F16, kind="ExternalInput")
    edgesT_d = nc.dram_tensor("edgesT", [F_IN, E], F16, kind="ExternalInput")
    ownT_d = nc.dram_tensor("ownT", [F_IN, RI], F16, kind="ExternalInput")
    # small per-head tensors packed into one DMA: [aN | aE | WNT | WET]
    SPK = 4 * HC + 2 * HC * F_IN
    spk_d = nc.dram_tensor("smallpk", [D, SPK], F16, kind="ExternalInput")
    wnwe_d = nc.dram_tensor("WNWE16", [F_IN, 2 * HC * D], F16, kind="ExternalInput")
    out_d = nc.dram_tensor("out", [RI, D], F32, kind="ExternalOutput")

    Copy = mybir.ActivationFunctionType.Copy
    Exp = mybir.ActivationFunctionType.Exp

    with tile.TileContext(nc) as tc, ExitStack() as ctx:
        singles = ctx.enter_context(tc.tile_pool(name="singles", bufs=1))
        workd = ctx.enter_context(tc.tile_pool(name="workd", bufs=6))
        work = ctx.enter_context(tc.tile_pool(name="work", bufs=14))
        small = ctx.enter_context(tc.tile_pool(name="small", bufs=6))
        psum_ht = ctx.enter_context(tc.tile_pool(name="psum_ht", bufs=3, space="PSUM"))
        psum_acc = ctx.enter_context(tc.tile_pool(name="psum_acc", bufs=3, space="PSUM"))
        psum_misc = ctx.enter_context(tc.tile_pool(name="psum_misc", bufs=2, space="PSUM"))

        # ---- persistent SBUF arrays -------------------------------------
        maskN = singles.tile([128, NCH, RI], F16, tag="maskN")
        maskE = singles.tile([128, ECH, RI], F16, tag="maskE")
        nodesT = singles.tile([128, 2, N], F16, tag="nodesT")
        edgesT = singles.tile([128, 2, E], F16, tag="edgesT")
        ownT = singles.tile([128, 2, RI], F16, tag="ownT")
        wnwe = singles.tile([128, 2, 2 * HC * D], F16, tag="wnwe")
        spk = singles.tile([D, SPK], F16, tag="spk")
        aN = spk[:, 0:3 * HC]
        aE = spk[:, 3 * HC:4 * HC]
        WNT = spk[:, 4 * HC:4 * HC + HC * F_IN].rearrange("d (h k) -> d h k", h=HC)
        WET = spk[:, 4 * HC + HC * F_IN:SPK].rearrange("d (h k) -> d h k", h=HC)
        WN = wnwe[:, :, 0:HC * D]
        WE = wnwe[:, :, HC * D:2 * HC * D]
        # per-j exponential vectors (j-chunk partition layout)
        e10n = singles.tile([128, NCH, 3 * HC], F32, tag="e10n")
        e2n = singles.tile([128, NCH, 3 * HC], F32, tag="e2n")
        e10e = singles.tile([128, ECH, HC], F32, tag="e10e")
        e2e = singles.tile([128, ECH, HC], F32, tag="e2e")
        # V tiles: [ht | 1] per (j-chunk, head); 66-wide for 4B alignment
        VT = singles.tile([128, NCH + ECH, HC, 66], F16, tag="VT")
        # P' broadcast tiles per (head, part): [j-partition-bcast, RI]
        Pb = singles.tile([128, HC, 2, RI], F16, tag="Pb")
        ptsb = singles.tile([U, IC, 128], F16, tag="ptsb")
        selU = singles.tile([U, U * 128], F16, tag="selU")
        identF16 = singles.tile([128, 128], F16, tag="identF16")
        acc = singles.tile([128, IC, D], F32, tag="acc")

        # ---- input DMAs (critical-path tensors first) --------------------
        # issue DMAs from three queues in parallel: SP carries the score-path
        # tensors, the idle-at-start Act queue the V-path ones, and the Tensor
        # queue the masks.
        nT = nodesT_d.rearrange("(c p) n -> p c n", p=128)
        mN = maskN_d.rearrange("(t p) i -> p t i", p=128)
        mE = maskE_d.rearrange("(t p) i -> p t i", p=128)
        nc.sync.dma_start(out=spk, in_=spk_d[:, :])
        nc.sync.dma_start(out=ownT, in_=ownT_d.rearrange("(c p) n -> p c n", p=128))
        nc.sync.dma_start(out=nodesT[:, :, 0:N // 2], in_=nT[:, :, 0:N // 2])
        nc.sync.dma_start(out=maskN[:, 0:4, :], in_=mN[:, 0:4, :])
        nc.sync.dma_start(out=nodesT[:, :, N // 2:N], in_=nT[:, :, N // 2:N])
        nc.sync.dma_start(out=wnwe, in_=wnwe_d.rearrange("(c p) m -> p c m", p=128))
        nc.sync.dma_start(out=maskN[:, 4:NCH, :], in_=mN[:, 4:NCH, :])
        nc.sync.dma_start(out=edgesT, in_=edgesT_d.rearrange("(c p) n -> p c n", p=128))
        nc.sync.dma_start(out=maskE[:, 0:16, :], in_=mE[:, 0:16, :])
        nc.sync.dma_start(out=maskE[:, 16:ECH, :], in_=mE[:, 16:ECH, :])

        # preload the Exp activation table while DMAs run
        warm = singles.tile([1, 1], F32, tag="warm")
        nc.vector.memset(warm, 0.0)
        nc.scalar.activation(warm[:, :], warm[:, :], Exp)

        make_identity(nc, identF16)
        nc.vector.memset(acc, 0.0)
        nc.gpsimd.memset(selU, 0.0)
        nc.gpsimd.affine_select(
            out=selU, in_=selU, compare_op=mybir.AluOpType.not_equal,
            fill=1.0, base=0, pattern=[[-1, U], [0, 128]],
            channel_multiplier=1,
        )
        # ones-column scaled by H so the softmax denominator carries the 1/H
        # head-mean for free: contrib = S[:,0:64] * recip(H * denom)
        nc.vector.memset(VT[:, :, :, 64:66], 0.0)
        nc.vector.memset(VT[:, :, :, 64:65], float(H))

        # ---- attention weight vectors: wv[k, v] = sum_d W[k, hd] a[h, d] --
        wvN = singles.tile([128, 2, 3 * HC], F16, tag="wvN")
        wvE = singles.tile([128, 2, HC], F16, tag="wvE")
        pwv = psum_misc.tile([128, 2, 3 * HC], F32, tag="pm")
        pwe = psum_misc.tile([128, 2, HC], F32, tag="pm")
        for kc in range(2):
            for h in range(HC):
                nc.tensor.matmul(pwv[:, kc, 3 * h:3 * h + 3],
                                 WNT[:, h, kc * 128:(kc + 1) * 128],
                                 aN[:, 3 * h:3 * h + 3])
                nc.tensor.matmul(pwe[:, kc, h:h + 1],
                                 WET[:, h, kc * 128:(kc + 1) * 128],
                                 aE[:, h:h + 1])
        nc.vector.tensor_copy(wvN[:, :, :], pwv[:, :, :])
        nc.vector.tensor_copy(wvE[:, :, :], pwe[:, :, :])

        # ---- own-row P' = e^{0.8 src} -> broadcast tiles ------------------
        e8own = small.tile([128, IC, 3 * HC], F16, tag="e8own")
        pso = psum_misc.tile([128, IC, 3 * HC], F32, tag="pm")
        for ch in range(IC):
            for kc in range(2):
                nc.tensor.matmul(pso[:, ch, :],
                                 ownT[:, kc, ch * 128:(ch + 1) * 128],
                                 wvN[:, kc, :], start=(kc == 0), stop=(kc == 1))
        nc.scalar.activation(e8own[:, :, :], pso[:, :, :], Exp, scale=0.8)
        # gather the 2*HC needed src columns (u = 2h+part <- col 3h+2*part),
        # transpose to rows, broadcast each row across partitions via selU.
        e8cols = small.tile([128, IC, U], F16, tag="e8cols")
        e8all = e8own[:, :, :]
        cols = bass.AP(tensor=e8all.tensor, offset=e8all.offset,
                       ap=[e8all.ap[0], [3 * HC, IC], [3, HC], [2, 2]])
        nc.vector.tensor_copy(e8cols[:, :, :], cols)
        pt = psum_misc.tile([U, IC, 128], F16, tag="pm")
        for ch in range(IC):
            nc.tensor.transpose(pt[:, ch, :], e8cols[:, ch, :], identF16[:, :])
        nc.vector.tensor_copy(ptsb[:, :, :], pt[:, :, :])
        # ---- first-head P' broadcast (fast path), then scores/exps, then
        # the remaining broadcasts on the Act engine ------------------------
        def emit_pb(u):
            h, part = u // 2, u % 2
            pb = psum_misc.tile([128, RI], F32, tag="pm")
            nc.tensor.matmul(pb[:, :], selU[:, u * 128:(u + 1) * 128],
                             ptsb[:, :, :].rearrange("u c p -> u (c p)"))
            if u < 2:
                nc.vector.tensor_copy(Pb[:, h, part, :], pb[:, :])
            else:
                nc.scalar.activation(Pb[:, h, part, :], pb[:, :], Copy)

        emit_pb(0)
        emit_pb(1)

        # ---- src/tgt scores -> per-j exponentials ------------------------
        psn = psum_misc.tile([128, NCH, 3 * HC], F32, tag="pm")
        for ch in range(NCH):
            for kc in range(2):
                nc.tensor.matmul(psn[:, ch, :],
                                 nodesT[:, kc, ch * 128:(ch + 1) * 128],
                                 wvN[:, kc, :], start=(kc == 0), stop=(kc == 1))
            if ch == 7:
                nc.scalar.activation(e10n[:, 0:8, :], psn[:, 0:8, :], Exp, scale=1.0)
                nc.scalar.activation(e2n[:, 0:8, :], psn[:, 0:8, :], Exp, scale=0.2)
        nc.scalar.activation(e10n[:, 8:NCH, :], psn[:, 8:NCH, :], Exp, scale=1.0)
        nc.scalar.activation(e2n[:, 8:NCH, :], psn[:, 8:NCH, :], Exp, scale=0.2)

        for u in range(2, U):
            emit_pb(u)

        # ---- ht = emb @ W, stored as [ht | 1] fp16 V tiles ---------------
        def emit_ht(ch):
            ph = psum_ht.tile([128, HC * D], F32, tag="ph")
            for kc in range(2):
                if ch < NCH:
                    lhsT = nodesT[:, kc, ch * 128:(ch + 1) * 128]
                    rhs = WN[:, kc, :]
                else:
                    lhsT = edgesT[:, kc, (ch - NCH) * 128:(ch - NCH + 1) * 128]
                    rhs = WE[:, kc, :]
                nc.tensor.matmul(ph[:, :], lhsT, rhs, start=(kc == 0), stop=(kc == 1))
            nc.scalar.activation(
                VT[:, ch, :, 0:64],
                ph[:, :].rearrange("p (h d) -> p h d", h=HC),
                Copy,
            )

        for ch in range(NCH):
            emit_ht(ch)

        def emit_edges_prep():
            pse = psum_misc.tile([128, ECH, HC], F32, tag="pm")
            for ch in range(ECH):
                for kc in range(2):
                    nc.tensor.matmul(pse[:, ch, :],
                                     edgesT[:, kc, ch * 128:(ch + 1) * 128],
                                     wvE[:, kc, :], start=(kc == 0), stop=(kc == 1))
            nc.scalar.activation(e10e[:, :, :], pse[:, :, :], Exp, scale=1.0)
            nc.scalar.activation(e2e[:, :, :], pse[:, :, :], Exp, scale=0.2)
            for ch in range(NCH, NCH + ECH):
                emit_ht(ch)

        # ---- main loop ---------------------------------------------------
        # u = mask * max(P'*QT, T); S^T[i, 0:65] += ut-chunk^T contract V65.
        # The mask multiply is split per group: first G-GPC chunks on DVE,
        # last GPC chunks on GpSimd, so both engines fill in parallel and the
        # in-order PE sees the DVE-made chunks first.
        for part in range(2):
            njt = NCH if part == 0 else ECH
            for h in range(HC):
                if part == 0 and h == 1:
                    emit_edges_prep()
                Sp = psum_acc.tile([128, IC, 65], F32, tag="Sacc")
                for gi, jt0 in enumerate(range(0, njt, G)):
                    dt_ = workd.tile([128, G, RI], F16, tag="Dt")
                    gpts = (part == 0 and gi % 2 == 0) or (part == 1 and gi == 0)
                    for g in range(G):
                        jt = jt0 + g
                        if part == 0:
                            q10 = e10n[:, jt, 3 * h + 1:3 * h + 2]
                            q2 = e2n[:, jt, 3 * h + 1:3 * h + 2]
                        else:
                            q10 = e10e[:, jt, h:h + 1]
                            q2 = e2e[:, jt, h:h + 1]
                        # for alternating N-groups the gp-owned chunk's score
                        # op also runs on GpSimd, keeping its chain on-engine
                        eng = nc.gpsimd if (gpts and g == G - 1) else nc.vector
                        eng.tensor_scalar(
                            out=dt_[:, g, :], in0=Pb[:, h, part, :],
                            scalar1=q10, scalar2=q2,
                            op0=mybir.AluOpType.mult, op1=mybir.AluOpType.max,
                        )
                    ut = work.tile([128, G, RI], F16, tag="ut")
                    msk = maskN if part == 0 else maskE
                    nd = G - (GPC_N if part == 0 else GPC_E)[gi]
                    if nd > 0:
                        nc.vector.tensor_mul(ut[:, 0:nd, :], dt_[:, 0:nd, :],
                                             msk[:, jt0:jt0 + nd, :])
                    if nd < G:
                        nc.gpsimd.tensor_mul(ut[:, nd:G, :], dt_[:, nd:G, :],
                                             msk[:, jt0 + nd:jt0 + G, :])
                    for g in range(G):
                        jt = jt0 + g
                        vch = jt if part == 0 else NCH + jt
                        # one start marks Sp's whole PSUM zero-region pending:
                        # each ic slice's first write then overwrites, later
                        # writes accumulate (per-ic starts would re-mark and
                        # wipe sibling slices' first chunk).
                        for ic in range(IC):
                            nc.tensor.matmul(
                                Sp[:, ic, :],
                                ut[:, g, ic * 128:(ic + 1) * 128],
                                VT[:, vch, h, 0:65],
                                start=(jt == 0 and ic == 0),
                                stop=(jt == njt - 1 and ic == IC - 1),
                            )
                # ---- normalize + accumulate ------------------------------
                contrib = small.tile([128, IC, D], F32, tag="contrib")
                rec = small.tile([128, IC, 1], F32, tag="rec")
                nc.vector.reciprocal(rec[:, :, :], Sp[:, :, 64:65])
                last = part == 1 and h == HC - 1
                for ic in range(IC):
                    if last:
                        # all-DVE tail: avoids cross-engine sem hops at the end
                        nc.vector.tensor_scalar(
                            out=contrib[:, ic, :], in0=Sp[:, ic, 0:64],
                            scalar1=rec[:, ic, :], scalar2=None,
                            op0=mybir.AluOpType.mult)
                    else:
                        nc.scalar.activation(contrib[:, ic, :],
                                             Sp[:, ic, 0:64], Copy,
                                             scale=rec[:, ic, :])
                if last:
                    oD = out_d.rearrange("(c p) d -> p c d", p=128)
                    for half in range(2):
                        i0, i1 = half * (IC // 2), (half + 1) * (IC // 2)
                        nc.vector.tensor_add(acc[:, i0:i1, :], acc[:, i0:i1, :],
                                             contrib[:, i0:i1, :])
                        nc.sync.dma_start(out=oD[:, i0:i1, :],
                                          in_=acc[:, i0:i1, :])
                else:
                    nc.gpsimd.tensor_add(acc[:, :, :], acc[:, :, :],
                                         contrib[:, :, :])

    return nc


def _get_program() -> bass.Bass:
    global _PROGRAM
    if _PROGRAM is None:
        nc = _build_program()
        nc.finalize()
        _PROGRAM = nc
    return _PROGRAM


def _prepare_in_maps(inputs) -> list:
    nodes = np.ascontiguousarray(np.asarray(inputs["nodes_embeddings"], np.float32))
    edges = np.ascontiguousarray(np.asarray(inputs["edges_embeddings"], np.float32))
    WNf = np.asarray(inputs["WN"], np.float32)
    WEf = np.asarray(inputs["WE"], np.float32)
    aNf = np.asarray(inputs["aN"], np.float32)
    aEf = np.asarray(inputs["aE"], np.float32)
    mat_nodes = np.asarray(inputs["mat_nodes"])
    mat_edges = np.asarray(inputs["mat_edges"])

    f16 = np.float16
    nodesT16 = np.ascontiguousarray(nodes.T.astype(f16))
    edgesT16 = np.ascontiguousarray(edges.T.astype(f16))
    maskN_T = np.ascontiguousarray(mat_nodes.astype(f16).T)  # [j, i_global]
    maskE_T = np.ascontiguousarray(mat_edges.astype(f16).T)

    # per head-shard weight slices
    WN_sh, WE_sh, WNT_sh, WET_sh, aN_sh, aE_sh = [], [], [], [], [], []
    for sh in range(SH):
        hs = range(sh * HC, (sh + 1) * HC)
        cols = slice(sh * HC * D, (sh + 1) * HC * D)
        WNg = WNf[:, cols]
        WEg = WEf[:, cols]
        WN_sh.append(WNg.astype(f16))
        WE_sh.append(WEg.astype(f16))
        WNT_sh.append(np.ascontiguousarray(
            WNg.T.astype(f16).reshape(HC, D, F_IN).transpose(1, 0, 2).reshape(D, HC * F_IN)))
        WET_sh.append(np.ascontiguousarray(
            WEg.T.astype(f16).reshape(HC, D, F_IN).transpose(1, 0, 2).reshape(D, HC * F_IN)))
        aN16 = np.empty((D, 3 * HC), f16)
        aE16 = np.empty((D, HC), f16)
        for k, h in enumerate(hs):
            aN16[:, 3 * k] = aNf[h, :D, 0].astype(f16)
            aN16[:, 3 * k + 1] = aNf[h, D:, 0].astype(f16)
            aN16[:, 3 * k + 2] = aEf[h, :D, 0].astype(f16)
            aE16[:, k] = aEf[h, D:, 0].astype(f16)
        aN_sh.append(aN16)
        aE_sh.append(aE16)

    in_maps = []
    for c in range(NCORES):
        si, sh = c // SH, c % SH
        sl = slice(si * RI, (si + 1) * RI)
        in_maps.append({
            "maskN": np.ascontiguousarray(maskN_T[:, sl]),
            "maskE": np.ascontiguousarray(maskE_T[:, sl]),
            "nodesT": nodesT16,
            "edgesT": edgesT16,
            "ownT": np.ascontiguousarray(nodesT16[:, sl]),
            "WNWE16": np.ascontiguousarray(
                np.concatenate([WN_sh[sh], WE_sh[sh]], axis=1)),
            "smallpk": np.ascontiguousarray(np.concatenate(
                [aN_sh[sh], aE_sh[sh], WNT_sh[sh], WET_sh[sh]], axis=1)),
        })
    return in_maps


def kernel(**inputs) -> np.ndarray:
    in_maps = _prepare_in_maps(inputs)
    nc = _get_program()
    res = run_bass_kernel_spmd(nc, in_maps, core_ids=list(range(NCORES)))
    parts = []
    for si in range(SI):
        acc = res.results[si * SH]["out"].astype(np.float32)
        for sh in range(1, SH):
            acc = acc + res.results[si * SH + sh]["out"]
        parts.append(acc)
    return np.concatenate(parts, axis=0)


# revision 10
# speedup vs baseline: 1.0027x; 1.0027x over previous
"""GAT (graph attention) kernel for Trainium2, 8 NeuronCores, v2.

Math identical to the baseline kernel: per head h,
    u[j,i] = mask[j,i] * max(P'_i * QT_j, T_j)
with P'_i = e^{0.8 src_i}, QT_j = e^{tgt_j}, T_j = e^{0.2 tgt_j}; the row
factor e^{0.2 src_i} cancels in the softmax.  Denominator comes from a
ones-column in V.

v2 changes vs baseline:
  * 2D sharding: SI i-shards x SH head-shards (SI*SH = 8 cores).  Each core
    owns RI = N/SI destination rows and HC = H/SH heads; the host adds the
    SH partial outputs per i-slice (mean over heads is linear).
  * Transposed attention matmuls: out[i,65] = ut[j,i]^T-contract V65[j,65].
    Output free size 65 instead of R -> ~2x less PE time.
  * Mask multiplies split between DVE and GpSimd (Pool) so the two engines
    work in parallel; everything else identical math.
"""

import numpy as np

from contextlib import ExitStack

import concourse.bass as bass
import concourse.bacc as bacc
import concourse.mybir as mybir
import concourse.tile as tile
from concourse.bass_utils import run_bass_kernel_spmd
from concourse.masks import make_identity

N, E, F_IN, H, D = 2048, 4096, 256, 8, 64
NCORES = 8
SI = 4                   # i-shards (destination-row shards)
SH = NCORES // SI        # head shards
RI = N // SI             # destination rows per core
IC = RI // 128           # i chunks of 128
HC = H // SH             # heads per core
NCH = N // 128           # 16 node j-chunks
ECH = E // 128           # 32 edge j-chunks
U = 2 * HC               # (head, part) broadcast rows
G = 4                    # j-chunks per fused mask-multiply group
F16 = mybir.dt.float16
F32 = mybir.dt.float32

# chunks per mask-multiply group handed to GpSimd, per group index within a
# (part, head) sweep: front-loaded so the Pool engine's backlog drains before
# the head's last (PSUM-stopping) matmuls and never runs past DVE at the tail.
GPC_N = [1, 1, 1, 1]
GPC_E = [2, 2, 2, 1, 1, 1, 1, 0]

_PROGRAM = None


def _build_program() -> bass.Bass:
    nc = bacc.Bacc("TRN2", target_bir_lowering=False, debug=False)

    maskN_d = nc.dram_tensor("maskN", [N, RI], F16, kind="ExternalInput")
    maskE_d = nc.dram_tensor("maskE", [E, RI], F16, kind="ExternalInput")
    nodesT_d = nc.dram_tensor("nodesT", [F_IN, N], F16, kind="ExternalInput")
    edgesT_d = nc.dram_tensor("edgesT", [F_IN, E], F16, kind="ExternalInput")
    ownT_d = nc.dram_tensor("ownT", [F_IN, RI], F16, kind="ExternalInput")
    # small per-head tensors packed into one DMA: [aN | aE | WNT | WET]
    SPK = 4 * HC + 2 * HC * F_IN
    spk_d = nc.dram_tensor("smallpk", [D, SPK], F16, kind="ExternalInput")
    wnwe_d = nc.dram_tensor("WNWE16", [F_IN, 2 * HC * D], F16, kind="ExternalInput")
    out_d = nc.dram_tensor("out", [RI, D], F32, kind="ExternalOutput")

    Copy = mybir.ActivationFunctionType.Copy
    Exp = mybir.ActivationFunctionType.Exp

    with tile.TileContext(nc) as tc, ExitStack() as ctx:
        singles = ctx.enter_context(tc.tile_pool(name="singles", bufs=1))
        workd = ctx.enter_context(tc.tile_pool(name="workd", bufs=6))
        work = ctx.enter_context(tc.tile_pool(name="work", bufs=14))
        small = ctx.enter_context(tc.tile_pool(name="small", bufs=6))
        psum_ht = ctx.enter_context(tc.tile_pool(name="psum_ht", bufs=3, space="PSUM"))
        psum_acc = ctx.enter_context(tc.tile_pool(name="psum_acc", bufs=3, space="PSUM"))
        psum_misc = ctx.enter_context(tc.tile_pool(name="psum_misc", bufs=2, space="PSUM"))

        # ---- persistent SBUF arrays -------------------------------------
        maskN = singles.tile([128, NCH, RI], F16, tag="maskN")
        maskE = singles.tile([128, ECH, RI], F16, tag="maskE")
        nodesT = singles.tile([128, 2, N], F16, tag="nodesT")
        edgesT = singles.tile([128, 2, E], F16, tag="edgesT")
        ownT = singles.tile([128, 2, RI], F16, tag="ownT")
        wnwe = singles.tile([128, 2, 2 * HC * D], F16, tag="wnwe")
        spk = singles.tile([D, SPK], F16, tag="spk")
        aN = spk[:, 0:3 * HC]
        aE = spk[:, 3 * HC:4 * HC]
        WNT = spk[:, 4 * HC:4 * HC + HC * F_IN].rearrange("d (h k) -> d h k", h=HC)
        WET = spk[:, 4 * HC + HC * F_IN:SPK].rearrange("d (h k) -> d h k", h=HC)
        WN = wnwe[:, :, 0:HC * D]
        WE = wnwe[:, :, HC * D:2 * HC * D]
        # per-j exponential vectors (j-chunk partition layout)
        e10n = singles.tile([128, NCH, 3 * HC], F32, tag="e10n")
        e2n = singles.tile([128, NCH, 3 * HC], F32, tag="e2n")
        e10e = singles.tile([128, ECH, HC], F32, tag="e10e")
        e2e = singles.tile([128, ECH, HC], F32, tag="e2e")
        # V tiles: [ht | 1] per (j-chunk, head); 66-wide for 4B alignment
        VT = singles.tile([128, NCH + ECH, HC, 66], F16, tag="VT")
        # P' broadcast tiles per (head, part): [j-partition-bcast, RI]
        Pb = singles.tile([128, HC, 2, RI], F16, tag="Pb")
        ptsb = singles.tile([U, RI], F16, tag="ptsb")
        selU = singles.tile([U, U * 128], F16, tag="selU")
        acc = singles.tile([128, IC, D], F32, tag="acc")

        # ---- input DMAs (critical-path tensors first) --------------------
        # issue DMAs from three queues in parallel: SP carries the score-path
        # tensors, the idle-at-start Act queue the V-path ones, and the Tensor
        # queue the masks.
        nT = nodesT_d.rearrange("(c p) n -> p c n", p=128)
        mN = maskN_d.rearrange("(t p) i -> p t i", p=128)
        mE = maskE_d.rearrange("(t p) i -> p t i", p=128)
        nc.sync.dma_start(out=spk, in_=spk_d[:, :])
        nc.sync.dma_start(out=ownT, in_=ownT_d.rearrange("(c p) n -> p c n", p=128))
        nc.sync.dma_start(out=nodesT[:, :, 0:N // 2], in_=nT[:, :, 0:N // 2])
        nc.sync.dma_start(out=maskN[:, 0:4, :], in_=mN[:, 0:4, :])
        nc.sync.dma_start(out=nodesT[:, :, N // 2:N], in_=nT[:, :, N // 2:N])
        nc.sync.dma_start(out=wnwe, in_=wnwe_d.rearrange("(c p) m -> p c m", p=128))
        nc.sync.dma_start(out=maskN[:, 4:NCH, :], in_=mN[:, 4:NCH, :])
        nc.sync.dma_start(out=edgesT, in_=edgesT_d.rearrange("(c p) n -> p c n", p=128))
        nc.sync.dma_start(out=maskE[:, 0:16, :], in_=mE[:, 0:16, :])
        nc.sync.dma_start(out=maskE[:, 16:ECH, :], in_=mE[:, 16:ECH, :])

        # preload the Exp activation table while DMAs run
        warm = singles.tile([1, 1], F32, tag="warm")
        nc.vector.memset(warm, 0.0)
        nc.scalar.activation(warm[:, :], warm[:, :], Exp)

        nc.vector.memset(acc, 0.0)
        nc.gpsimd.memset(selU, 0.0)
        nc.gpsimd.affine_select(
            out=selU, in_=selU, compare_op=mybir.AluOpType.not_equal,
            fill=1.0, base=0, pattern=[[-1, U], [0, 128]],
            channel_multiplier=1,
        )
        # ones-column scaled by H so the softmax denominator carries the 1/H
        # head-mean for free: contrib = S[:,0:64] * recip(H * denom)
        nc.vector.memset(VT[:, :, :, 64:66], 0.0)
        nc.vector.memset(VT[:, :, :, 64:65], float(H))

        # ---- attention weight vectors: wv[k, v] = sum_d W[k, hd] a[h, d] --
        wvN = singles.tile([128, 2, 3 * HC], F16, tag="wvN")
        wvE = singles.tile([128, 2, HC], F16, tag="wvE")
        pwv = psum_misc.tile([128, 2, 3 * HC], F32, tag="pm")
        pwe = psum_misc.tile([128, 2, HC], F32, tag="pm")
        for kc in range(2):
            for h in range(HC):
                nc.tensor.matmul(pwv[:, kc, 3 * h:3 * h + 3],
                                 WNT[:, h, kc * 128:(kc + 1) * 128],
                                 aN[:, 3 * h:3 * h + 3])
                nc.tensor.matmul(pwe[:, kc, h:h + 1],
                                 WET[:, h, kc * 128:(kc + 1) * 128],
                                 aE[:, h:h + 1])
        nc.vector.tensor_copy(wvN[:, :, :], pwv[:, :, :])
        nc.vector.tensor_copy(wvE[:, :, :], pwe[:, :, :])

        # ---- own-row P' = e^{0.8 src} -> broadcast tiles ------------------
        # compute the own scores already TRANSPOSED on the PE (rows u =
        # (head, part), gathered attention vectors as lhsT), then exp the
        # rows directly -- no per-chunk transposes needed.
        wvsel = small.tile([128, 2, U], F16, tag="wvsel")
        wvall = wvN[:, :, :]
        wcols = bass.AP(tensor=wvall.tensor, offset=wvall.offset,
                        ap=[wvall.ap[0], [3 * HC, 2], [3, HC], [2, 2]])
        nc.vector.tensor_copy(wvsel[:, :, :], wcols)
        pscT = psum_misc.tile([U, RI], F32, tag="pm")
        for kc in range(2):
            nc.tensor.matmul(pscT[:, :], wvsel[:, kc, :], ownT[:, kc, :],
                             start=(kc == 0), stop=(kc == 1))
        nc.scalar.activation(ptsb[:, :], pscT[:, :], Exp, scale=0.8)
        # ---- first-head P' broadcast (fast path), then scores/exps, then
        # the remaining broadcasts on the Act engine ------------------------
        def emit_pb(u):
            h, part = u // 2, u % 2
            pb = psum_misc.tile([128, RI], F32, tag="pm")
            nc.tensor.matmul(pb[:, :], selU[:, u * 128:(u + 1) * 128],
                             ptsb[:, :])
            if u < 2:
                nc.vector.tensor_copy(Pb[:, h, part, :], pb[:, :])
            else:
                nc.scalar.activation(Pb[:, h, part, :], pb[:, :], Copy)

        emit_pb(0)
        emit_pb(1)

        # ---- src/tgt scores -> per-j exponentials ------------------------
        psn = psum_misc.tile([128, NCH, 3 * HC], F32, tag="pm")
        for ch in range(NCH):
            for kc in range(2):
                nc.tensor.matmul(psn[:, ch, :],
                                 nodesT[:, kc, ch * 128:(ch + 1) * 128],
                                 wvN[:, kc, :], start=(kc == 0), stop=(kc == 1))
            if ch == 7:
                nc.scalar.activation(e10n[:, 0:8, :], psn[:, 0:8, :], Exp, scale=1.0)
                nc.scalar.activation(e2n[:, 0:8, :], psn[:, 0:8, :], Exp, scale=0.2)
        nc.scalar.activation(e10n[:, 8:NCH, :], psn[:, 8:NCH, :], Exp, scale=1.0)
        nc.scalar.activation(e2n[:, 8:NCH, :], psn[:, 8:NCH, :], Exp, scale=0.2)

        for u in range(2, U):
            emit_pb(u)

        # ---- ht = emb @ W, stored as [ht | 1] fp16 V tiles ---------------
        def emit_ht(ch):
            ph = psum_ht.tile([128, HC * D], F32, tag="ph")
            for kc in range(2):
                if ch < NCH:
                    lhsT = nodesT[:, kc, ch * 128:(ch + 1) * 128]
                    rhs = WN[:, kc, :]
                else:
                    lhsT = edgesT[:, kc, (ch - NCH) * 128:(ch - NCH + 1) * 128]
                    rhs = WE[:, kc, :]
                nc.tensor.matmul(ph[:, :], lhsT, rhs, start=(kc == 0), stop=(kc == 1))
            nc.scalar.activation(
                VT[:, ch, :, 0:64],
                ph[:, :].rearrange("p (h d) -> p h d", h=HC),
                Copy,
            )

        for ch in range(NCH):
            emit_ht(ch)

        def emit_edges_prep():
            pse = psum_misc.tile([128, ECH, HC], F32, tag="pm")
            for ch in range(ECH):
                for kc in range(2):
                    nc.tensor.matmul(pse[:, ch, :],
                                     edgesT[:, kc, ch * 128:(ch + 1) * 128],
                                     wvE[:, kc, :], start=(kc == 0), stop=(kc == 1))
            nc.scalar.activation(e10e[:, :, :], pse[:, :, :], Exp, scale=1.0)
            nc.scalar.activation(e2e[:, :, :], pse[:, :, :], Exp, scale=0.2)
            for ch in range(NCH, NCH + ECH):
                emit_ht(ch)

        # ---- main loop ---------------------------------------------------
        # u = mask * max(P'*QT, T); S^T[i, 0:65] += ut-chunk^T contract V65.
        # The mask multiply is split per group: first G-GPC chunks on DVE,
        # last GPC chunks on GpSimd, so both engines fill in parallel and the
        # in-order PE sees the DVE-made chunks first.
        for part in range(2):
            njt = NCH if part == 0 else ECH
            for h in range(HC):
                if part == 0 and h == 1:
                    emit_edges_prep()
                Sp = psum_acc.tile([128, IC, 65], F32, tag="Sacc")
                for gi, jt0 in enumerate(range(0, njt, G)):
                    dt_ = workd.tile([128, G, RI], F16, tag="Dt")
                    gpts = (part == 0 and gi % 2 == 0) or (part == 1 and gi == 0)
                    for g in range(G):
                        jt = jt0 + g
                        if part == 0:
                            q10 = e10n[:, jt, 3 * h + 1:3 * h + 2]
                            q2 = e2n[:, jt, 3 * h + 1:3 * h + 2]
                        else:
                            q10 = e10e[:, jt, h:h + 1]
                            q2 = e2e[:, jt, h:h + 1]
                        # for alternating N-groups the gp-owned chunk's score
                        # op also runs on GpSimd, keeping its chain on-engine
                        eng = nc.gpsimd if (gpts and g == G - 1) else nc.vector
                        eng.tensor_scalar(
                            out=dt_[:, g, :], in0=Pb[:, h, part, :],
                            scalar1=q10, scalar2=q2,
                            op0=mybir.AluOpType.mult, op1=mybir.AluOpType.max,
                        )
                    ut = work.tile([128, G, RI], F16, tag="ut")
                    msk = maskN if part == 0 else maskE
                    nd = G - (GPC_N if part == 0 else GPC_E)[gi]
                    if nd > 0:
                        nc.vector.tensor_mul(ut[:, 0:nd, :], dt_[:, 0:nd, :],
                                             msk[:, jt0:jt0 + nd, :])
                    if nd < G:
                        nc.gpsimd.tensor_mul(ut[:, nd:G, :], dt_[:, nd:G, :],
                                             msk[:, jt0 + nd:jt0 + G, :])
                    for g in range(G):
                        jt = jt0 + g
                        vch = jt if part == 0 else NCH + jt
                        # one start marks Sp's whole PSUM zero-region pending:
                        # each ic slice's first write then overwrites, later
                        # writes accumulate (per-ic starts would re-mark and
                        # wipe sibling slices' first chunk).
                        for ic in range(IC):
                            nc.tensor.matmul(
                                Sp[:, ic, :],
                                ut[:, g, ic * 128:(ic + 1) * 128],
                                VT[:, vch, h, 0:65],
                                start=(jt == 0 and ic == 0),
                                stop=(jt == njt - 1 and ic == IC - 1),
                            )
                # ---- normalize + accumulate ------------------------------
                contrib = small.tile([128, IC, D], F32, tag="contrib")
                rec = small.tile([128, IC, 1], F32, tag="rec")
                nc.vector.reciprocal(rec[:, :, :], Sp[:, :, 64:65])
                last = part == 1 and h == HC - 1
                for ic in range(IC):
                    if last:
                        # all-DVE tail: avoids cross-engine sem hops at the end
                        nc.vector.tensor_scalar(
                            out=contrib[:, ic, :], in0=Sp[:, ic, 0:64],
                            scalar1=rec[:, ic, :], scalar2=None,
                            op0=mybir.AluOpType.mult)
                    else:
                        nc.scalar.activation(contrib[:, ic, :],
                                             Sp[:, ic, 0:64], Copy,
                                             scale=rec[:, ic, :])
                if last:
                    oD = out_d.rearrange("(c p) d -> p c d", p=128)
                    for half in range(2):
                        i0, i1 = half * (IC // 2), (half + 1) * (IC // 2)
                        nc.vector.tensor_add(acc[:, i0:i1, :], acc[:, i0:i1, :],
                                             contrib[:, i0:i1, :])
                        nc.sync.dma_start(out=oD[:, i0:i1, :],
                                          in_=acc[:, i0:i1, :])
                else:
                    nc.gpsimd.tensor_add(acc[:, :, :], acc[:, :, :],
                                         contrib[:, :, :])

    return nc


def _get_program() -> bass.Bass:
    global _PROGRAM
    if _PROGRAM is None:
        nc = _build_program()
        nc.finalize()
        _PROGRAM = nc
    return _PROGRAM


def _prepare_in_maps(inputs) -> list:
    nodes = np.ascontiguousarray(np.asarray(inputs["nodes_embeddings"], np.float32))
    edges = np.ascontiguousarray(np.asarray(inputs["edges_embeddings"], np.float32))
    WNf = np.asarray(inputs["WN"], np.float32)
    WEf = np.asarray(inputs["WE"], np.float32)
    aNf = np.asarray(inputs["aN"], np.float32)
    aEf = np.asarray(inputs["aE"], np.float32)
    mat_nodes = np.asarray(inputs["mat_nodes"])
    mat_edges = np.asarray(inputs["mat_edges"])

    f16 = np.float16
    nodesT16 = np.ascontiguousarray(nodes.T.astype(f16))
    edgesT16 = np.ascontiguousarray(edges.T.astype(f16))
    maskN_T = np.ascontiguousarray(mat_nodes.astype(f16).T)  # [j, i_global]
    maskE_T = np.ascontiguousarray(mat_edges.astype(f16).T)

    # per head-shard weight slices
    WN_sh, WE_sh, WNT_sh, WET_sh, aN_sh, aE_sh = [], [], [], [], [], []
    for sh in range(SH):
        hs = range(sh * HC, (sh + 1) * HC)
        cols = slice(sh * HC * D, (sh + 1) * HC * D)
        WNg = WNf[:, cols]
        WEg = WEf[:, cols]
        WN_sh.append(WNg.astype(f16))
        WE_sh.append(WEg.astype(f16))
        WNT_sh.append(np.ascontiguousarray(
            WNg.T.astype(f16).reshape(HC, D, F_IN).transpose(1, 0, 2).reshape(D, HC * F_IN)))
        WET_sh.append(np.ascontiguousarray(
            WEg.T.astype(f16).reshape(HC, D, F_IN).transpose(1, 0, 2).reshape(D, HC * F_IN)))
        aN16 = np.empty((D, 3 * HC), f16)
        aE16 = np.empty((D, HC), f16)
        for k, h in enumerate(hs):
            aN16[:, 3 * k] = aNf[h, :D, 0].astype(f16)
            aN16[:, 3 * k + 1] = aNf[h, D:, 0].astype(f16)
            aN16[:, 3 * k + 2] = aEf[h, :D, 0].astype(f16)
            aE16[:, k] = aEf[h, D:, 0].astype(f16)
        aN_sh.append(aN16)
        aE_sh.append(aE16)

    in_maps = []
    for c in range(NCORES):
        si, sh = c // SH, c % SH
        sl = slice(si * RI, (si + 1) * RI)
        in_maps.append({
            "maskN": np.ascontiguousarray(maskN_T[:, sl]),
            "maskE": np.ascontiguousarray(maskE_T[:, sl]),
            "nodesT": nodesT16,
            "edgesT": edgesT16,
            "ownT": np.ascontiguousarray(nodesT16[:, sl]),
            "WNWE16": np.ascontiguousarray(
                np.concatenate([WN_sh[sh], WE_sh[sh]], axis=1)),
            "smallpk": np.ascontiguousarray(np.concatenate(
                [aN_sh[sh], aE_sh[sh], WNT_sh[sh], WET_sh[sh]], axis=1)),
        })
    return in_maps


def kernel(**inputs) -> np.ndarray:
    in_maps = _prepare_in_maps(inputs)
    nc = _get_program()
    res = run_bass_kernel_spmd(nc, in_maps, core_ids=list(range(NCORES)))
    parts = []
    for si in range(SI):
        acc = res.results[si * SH]["out"].astype(np.float32)
        for sh in range(1, SH):
            acc = acc + res.results[si * SH + sh]["out"]
        parts.append(acc)
    return np.concatenate(parts, axis=0)


# revision 11
# speedup vs baseline: 1.0053x; 1.0027x over previous
"""GAT (graph attention) kernel for Trainium2, 8 NeuronCores, v2.

Math identical to the baseline kernel: per head h,
    u[j,i] = mask[j,i] * max(P'_i * QT_j, T_j)
with P'_i = e^{0.8 src_i}, QT_j = e^{tgt_j}, T_j = e^{0.2 tgt_j}; the row
factor e^{0.2 src_i} cancels in the softmax.  Denominator comes from a
ones-column in V.

v2 changes vs baseline:
  * 2D sharding: SI i-shards x SH head-shards (SI*SH = 8 cores).  Each core
    owns RI = N/SI destination rows and HC = H/SH heads; the host adds the
    SH partial outputs per i-slice (mean over heads is linear).
  * Transposed attention matmuls: out[i,65] = ut[j,i]^T-contract V65[j,65].
    Output free size 65 instead of R -> ~2x less PE time.
  * Mask multiplies split between DVE and GpSimd (Pool) so the two engines
    work in parallel; everything else identical math.
"""

import numpy as np

from contextlib import ExitStack

import concourse.bass as bass
import concourse.bacc as bacc
import concourse.mybir as mybir
import concourse.tile as tile
from concourse.bass_utils import run_bass_kernel_spmd
from concourse.masks import make_identity

N, E, F_IN, H, D = 2048, 4096, 256, 8, 64
NCORES = 8
SI = 4                   # i-shards (destination-row shards)
SH = NCORES // SI        # head shards
RI = N // SI             # destination rows per core
IC = RI // 128           # i chunks of 128
HC = H // SH             # heads per core
NCH = N // 128           # 16 node j-chunks
ECH = E // 128           # 32 edge j-chunks
U = 2 * HC               # (head, part) broadcast rows
G = 4                    # j-chunks per fused mask-multiply group
F16 = mybir.dt.float16
F32 = mybir.dt.float32

# chunks per mask-multiply group handed to GpSimd, per group index within a
# (part, head) sweep: front-loaded so the Pool engine's backlog drains before
# the head's last (PSUM-stopping) matmuls and never runs past DVE at the tail.
GPC_N = [1, 1, 1, 1]
GPC_E = [2, 2, 2, 1, 1, 1, 1, 0]

_PROGRAM = None


def _build_program() -> bass.Bass:
    nc = bacc.Bacc("TRN2", target_bir_lowering=False, debug=False)

    maskN_d = nc.dram_tensor("maskN", [N, RI], F16, kind="ExternalInput")
    maskE_d = nc.dram_tensor("maskE", [E, RI], F16, kind="ExternalInput")
    nodesT_d = nc.dram_tensor("nodesT", [F_IN, N], F16, kind="ExternalInput")
    edgesT_d = nc.dram_tensor("edgesT", [F_IN, E], F16, kind="ExternalInput")
    ownT_d = nc.dram_tensor("ownT", [F_IN, RI], F16, kind="ExternalInput")
    # small per-head tensors packed into one DMA: [aN | aE | WNT | WET]
    SPK = 4 * HC + 2 * HC * F_IN
    spk_d = nc.dram_tensor("smallpk", [D, SPK], F16, kind="ExternalInput")
    wnwe_d = nc.dram_tensor("WNWE16", [F_IN, 2 * HC * D], F16, kind="ExternalInput")
    out_d = nc.dram_tensor("out", [RI, D], F32, kind="ExternalOutput")

    Copy = mybir.ActivationFunctionType.Copy
    Exp = mybir.ActivationFunctionType.Exp

    with tile.TileContext(nc) as tc, ExitStack() as ctx:
        singles = ctx.enter_context(tc.tile_pool(name="singles", bufs=1))
        workd = ctx.enter_context(tc.tile_pool(name="workd", bufs=20))
        small = ctx.enter_context(tc.tile_pool(name="small", bufs=6))
        psum_ht = ctx.enter_context(tc.tile_pool(name="psum_ht", bufs=3, space="PSUM"))
        psum_acc = ctx.enter_context(tc.tile_pool(name="psum_acc", bufs=3, space="PSUM"))
        psum_misc = ctx.enter_context(tc.tile_pool(name="psum_misc", bufs=2, space="PSUM"))

        # ---- persistent SBUF arrays -------------------------------------
        maskN = singles.tile([128, NCH, RI], F16, tag="maskN")
        maskE = singles.tile([128, ECH, RI], F16, tag="maskE")
        nodesT = singles.tile([128, 2, N], F16, tag="nodesT")
        edgesT = singles.tile([128, 2, E], F16, tag="edgesT")
        ownT = singles.tile([128, 2, RI], F16, tag="ownT")
        wnwe = singles.tile([128, 2, 2 * HC * D], F16, tag="wnwe")
        spk = singles.tile([D, SPK], F16, tag="spk")
        aN = spk[:, 0:3 * HC]
        aE = spk[:, 3 * HC:4 * HC]
        WNT = spk[:, 4 * HC:4 * HC + HC * F_IN].rearrange("d (h k) -> d h k", h=HC)
        WET = spk[:, 4 * HC + HC * F_IN:SPK].rearrange("d (h k) -> d h k", h=HC)
        WN = wnwe[:, :, 0:HC * D]
        WE = wnwe[:, :, HC * D:2 * HC * D]
        # per-j exponential vectors (j-chunk partition layout)
        e10n = singles.tile([128, NCH, 3 * HC], F32, tag="e10n")
        e2n = singles.tile([128, NCH, 3 * HC], F32, tag="e2n")
        e10e = singles.tile([128, ECH, HC], F32, tag="e10e")
        e2e = singles.tile([128, ECH, HC], F32, tag="e2e")
        # V tiles: [ht | 1] per (j-chunk, head); 66-wide for 4B alignment
        VT = singles.tile([128, NCH + ECH, HC, 66], F16, tag="VT")
        # P' broadcast tiles per (head, part): [j-partition-bcast, RI]
        Pb = singles.tile([128, HC, 2, RI], F16, tag="Pb")
        ptsb = singles.tile([U, RI], F16, tag="ptsb")
        selU = singles.tile([U, U * 128], F16, tag="selU")
        acc = singles.tile([128, IC, D], F32, tag="acc")

        # ---- input DMAs (critical-path tensors first) --------------------
        # issue DMAs from three queues in parallel: SP carries the score-path
        # tensors, the idle-at-start Act queue the V-path ones, and the Tensor
        # queue the masks.
        nT = nodesT_d.rearrange("(c p) n -> p c n", p=128)
        mN = maskN_d.rearrange("(t p) i -> p t i", p=128)
        mE = maskE_d.rearrange("(t p) i -> p t i", p=128)
        nc.sync.dma_start(out=spk, in_=spk_d[:, :])
        nc.sync.dma_start(out=ownT, in_=ownT_d.rearrange("(c p) n -> p c n", p=128))
        nc.sync.dma_start(out=nodesT[:, :, 0:N // 2], in_=nT[:, :, 0:N // 2])
        nc.sync.dma_start(out=maskN[:, 0:4, :], in_=mN[:, 0:4, :])
        nc.sync.dma_start(out=nodesT[:, :, N // 2:N], in_=nT[:, :, N // 2:N])
        nc.sync.dma_start(out=wnwe, in_=wnwe_d.rearrange("(c p) m -> p c m", p=128))
        nc.sync.dma_start(out=maskN[:, 4:NCH, :], in_=mN[:, 4:NCH, :])
        nc.sync.dma_start(out=edgesT, in_=edgesT_d.rearrange("(c p) n -> p c n", p=128))
        nc.sync.dma_start(out=maskE[:, 0:16, :], in_=mE[:, 0:16, :])
        nc.sync.dma_start(out=maskE[:, 16:ECH, :], in_=mE[:, 16:ECH, :])

        # preload the Exp activation table while DMAs run
        warm = singles.tile([1, 1], F32, tag="warm")
        nc.vector.memset(warm, 0.0)
        nc.scalar.activation(warm[:, :], warm[:, :], Exp)

        nc.vector.memset(acc, 0.0)
        nc.gpsimd.memset(selU, 0.0)
        nc.gpsimd.affine_select(
            out=selU, in_=selU, compare_op=mybir.AluOpType.not_equal,
            fill=1.0, base=0, pattern=[[-1, U], [0, 128]],
            channel_multiplier=1,
        )
        # ones-column scaled by H so the softmax denominator carries the 1/H
        # head-mean for free: contrib = S[:,0:64] * recip(H * denom)
        nc.vector.memset(VT[:, :, :, 64:66], 0.0)
        nc.vector.memset(VT[:, :, :, 64:65], float(H))

        # ---- attention weight vectors: wv[k, v] = sum_d W[k, hd] a[h, d] --
        wvN = singles.tile([128, 2, 3 * HC], F16, tag="wvN")
        wvE = singles.tile([128, 2, HC], F16, tag="wvE")
        pwv = psum_misc.tile([128, 2, 3 * HC], F32, tag="pm")
        pwe = psum_misc.tile([128, 2, HC], F32, tag="pm")
        for kc in range(2):
            for h in range(HC):
                nc.tensor.matmul(pwv[:, kc, 3 * h:3 * h + 3],
                                 WNT[:, h, kc * 128:(kc + 1) * 128],
                                 aN[:, 3 * h:3 * h + 3])
                nc.tensor.matmul(pwe[:, kc, h:h + 1],
                                 WET[:, h, kc * 128:(kc + 1) * 128],
                                 aE[:, h:h + 1])
        nc.vector.tensor_copy(wvN[:, :, :], pwv[:, :, :])
        nc.vector.tensor_copy(wvE[:, :, :], pwe[:, :, :])

        # ---- own-row P' = e^{0.8 src} -> broadcast tiles ------------------
        # compute the own scores already TRANSPOSED on the PE (rows u =
        # (head, part), gathered attention vectors as lhsT), then exp the
        # rows directly -- no per-chunk transposes needed.
        wvsel = small.tile([128, 2, U], F16, tag="wvsel")
        wvall = wvN[:, :, :]
        wcols = bass.AP(tensor=wvall.tensor, offset=wvall.offset,
                        ap=[wvall.ap[0], [3 * HC, 2], [3, HC], [2, 2]])
        nc.vector.tensor_copy(wvsel[:, :, :], wcols)
        pscT = psum_misc.tile([U, RI], F32, tag="pm")
        for kc in range(2):
            nc.tensor.matmul(pscT[:, :], wvsel[:, kc, :], ownT[:, kc, :],
                             start=(kc == 0), stop=(kc == 1))
        nc.scalar.activation(ptsb[:, :], pscT[:, :], Exp, scale=0.8)
        # ---- first-head P' broadcast (fast path), then scores/exps, then
        # the remaining broadcasts on the Act engine ------------------------
        def emit_pb(u):
            h, part = u // 2, u % 2
            pb = psum_misc.tile([128, RI], F32, tag="pm")
            nc.tensor.matmul(pb[:, :], selU[:, u * 128:(u + 1) * 128],
                             ptsb[:, :])
            if u < 2:
                nc.vector.tensor_copy(Pb[:, h, part, :], pb[:, :])
            else:
                nc.scalar.activation(Pb[:, h, part, :], pb[:, :], Copy)

        emit_pb(0)
        emit_pb(1)

        # ---- src/tgt scores -> per-j exponentials ------------------------
        psn = psum_misc.tile([128, NCH, 3 * HC], F32, tag="pm")
        for ch in range(NCH):
            for kc in range(2):
                nc.tensor.matmul(psn[:, ch, :],
                                 nodesT[:, kc, ch * 128:(ch + 1) * 128],
                                 wvN[:, kc, :], start=(kc == 0), stop=(kc == 1))
            if ch == 7:
                nc.scalar.activation(e10n[:, 0:8, :], psn[:, 0:8, :], Exp, scale=1.0)
                nc.scalar.activation(e2n[:, 0:8, :], psn[:, 0:8, :], Exp, scale=0.2)
        nc.scalar.activation(e10n[:, 8:NCH, :], psn[:, 8:NCH, :], Exp, scale=1.0)
        nc.scalar.activation(e2n[:, 8:NCH, :], psn[:, 8:NCH, :], Exp, scale=0.2)

        for u in range(2, U):
            emit_pb(u)

        # ---- ht = emb @ W, stored as [ht | 1] fp16 V tiles ---------------
        def emit_ht(ch):
            ph = psum_ht.tile([128, HC * D], F32, tag="ph")
            for kc in range(2):
                if ch < NCH:
                    lhsT = nodesT[:, kc, ch * 128:(ch + 1) * 128]
                    rhs = WN[:, kc, :]
                else:
                    lhsT = edgesT[:, kc, (ch - NCH) * 128:(ch - NCH + 1) * 128]
                    rhs = WE[:, kc, :]
                nc.tensor.matmul(ph[:, :], lhsT, rhs, start=(kc == 0), stop=(kc == 1))
            nc.scalar.activation(
                VT[:, ch, :, 0:64],
                ph[:, :].rearrange("p (h d) -> p h d", h=HC),
                Copy,
            )

        for ch in range(NCH):
            emit_ht(ch)

        def emit_edges_prep():
            pse = psum_misc.tile([128, ECH, HC], F32, tag="pm")
            for ch in range(ECH):
                for kc in range(2):
                    nc.tensor.matmul(pse[:, ch, :],
                                     edgesT[:, kc, ch * 128:(ch + 1) * 128],
                                     wvE[:, kc, :], start=(kc == 0), stop=(kc == 1))
            nc.scalar.activation(e10e[:, :, :], pse[:, :, :], Exp, scale=1.0)
            nc.scalar.activation(e2e[:, :, :], pse[:, :, :], Exp, scale=0.2)
            for ch in range(NCH, NCH + ECH):
                emit_ht(ch)

        # ---- main loop ---------------------------------------------------
        # u = mask * max(P'*QT, T); S^T[i, 0:65] += ut-chunk^T contract V65.
        # The mask multiply is split per group: first G-GPC chunks on DVE,
        # last GPC chunks on GpSimd, so both engines fill in parallel and the
        # in-order PE sees the DVE-made chunks first.
        for part in range(2):
            njt = NCH if part == 0 else ECH
            for h in range(HC):
                if part == 0 and h == 1:
                    emit_edges_prep()
                Sp = psum_acc.tile([128, IC, 65], F32, tag="Sacc")
                for gi, jt0 in enumerate(range(0, njt, G)):
                    dt_ = workd.tile([128, G, RI], F16, tag="Dt")
                    gpts = (part == 0 and gi % 2 == 0) or (part == 1 and gi == 0)
                    for g in range(G):
                        jt = jt0 + g
                        if part == 0:
                            q10 = e10n[:, jt, 3 * h + 1:3 * h + 2]
                            q2 = e2n[:, jt, 3 * h + 1:3 * h + 2]
                        else:
                            q10 = e10e[:, jt, h:h + 1]
                            q2 = e2e[:, jt, h:h + 1]
                        # for alternating N-groups the gp-owned chunk's score
                        # op also runs on GpSimd, keeping its chain on-engine
                        eng = nc.gpsimd if (gpts and g == G - 1) else nc.vector
                        eng.tensor_scalar(
                            out=dt_[:, g, :], in0=Pb[:, h, part, :],
                            scalar1=q10, scalar2=q2,
                            op0=mybir.AluOpType.mult, op1=mybir.AluOpType.max,
                        )
                    ut = dt_
                    msk = maskN if part == 0 else maskE
                    nd = G - (GPC_N if part == 0 else GPC_E)[gi]
                    if nd > 0:
                        nc.vector.tensor_mul(ut[:, 0:nd, :], dt_[:, 0:nd, :],
                                             msk[:, jt0:jt0 + nd, :])
                    if nd < G:
                        nc.gpsimd.tensor_mul(ut[:, nd:G, :], dt_[:, nd:G, :],
                                             msk[:, jt0 + nd:jt0 + G, :])
                    for g in range(G):
                        jt = jt0 + g
                        vch = jt if part == 0 else NCH + jt
                        # one start marks Sp's whole PSUM zero-region pending:
                        # each ic slice's first write then overwrites, later
                        # writes accumulate (per-ic starts would re-mark and
                        # wipe sibling slices' first chunk).
                        for ic in range(IC):
                            nc.tensor.matmul(
                                Sp[:, ic, :],
                                ut[:, g, ic * 128:(ic + 1) * 128],
                                VT[:, vch, h, 0:65],
                                start=(jt == 0 and ic == 0),
                                stop=(jt == njt - 1 and ic == IC - 1),
                            )
                # ---- normalize + accumulate ------------------------------
                contrib = small.tile([128, IC, D], F32, tag="contrib")
                rec = small.tile([128, IC, 1], F32, tag="rec")
                nc.vector.reciprocal(rec[:, :, :], Sp[:, :, 64:65])
                last = part == 1 and h == HC - 1
                for ic in range(IC):
                    if last:
                        # all-DVE tail: avoids cross-engine sem hops at the end
                        nc.vector.tensor_scalar(
                            out=contrib[:, ic, :], in0=Sp[:, ic, 0:64],
                            scalar1=rec[:, ic, :], scalar2=None,
                            op0=mybir.AluOpType.mult)
                    else:
                        nc.scalar.activation(contrib[:, ic, :],
                                             Sp[:, ic, 0:64], Copy,
                                             scale=rec[:, ic, :])
                if last:
                    oD = out_d.rearrange("(c p) d -> p c d", p=128)
                    for half in range(2):
                        i0, i1 = half * (IC // 2), (half + 1) * (IC // 2)
                        nc.vector.tensor_add(acc[:, i0:i1, :], acc[:, i0:i1, :],
                                             contrib[:, i0:i1, :])
                        nc.sync.dma_start(out=oD[:, i0:i1, :],
                                          in_=acc[:, i0:i1, :])
                else:
                    nc.gpsimd.tensor_add(acc[:, :, :], acc[:, :, :],
                                         contrib[:, :, :])

    return nc


def _get_program() -> bass.Bass:
    global _PROGRAM
    if _PROGRAM is None:
        nc = _build_program()
        nc.finalize()
        _PROGRAM = nc
    return _PROGRAM


def _prepare_in_maps(inputs) -> list:
    nodes = np.ascontiguousarray(np.asarray(inputs["nodes_embeddings"], np.float32))
    edges = np.ascontiguousarray(np.asarray(inputs["edges_embeddings"], np.float32))
    WNf = np.asarray(inputs["WN"], np.float32)
    WEf = np.asarray(inputs["WE"], np.float32)
    aNf = np.asarray(inputs["aN"], np.float32)
    aEf = np.asarray(inputs["aE"], np.float32)
    mat_nodes = np.asarray(inputs["mat_nodes"])
    mat_edges = np.asarray(inputs["mat_edges"])

    f16 = np.float16
    nodesT16 = np.ascontiguousarray(nodes.T.astype(f16))
    edgesT16 = np.ascontiguousarray(edges.T.astype(f16))
    maskN_T = np.ascontiguousarray(mat_nodes.astype(f16).T)  # [j, i_global]
    maskE_T = np.ascontiguousarray(mat_edges.astype(f16).T)

    # per head-shard weight slices
    WN_sh, WE_sh, WNT_sh, WET_sh, aN_sh, aE_sh = [], [], [], [], [], []
    for sh in range(SH):
        hs = range(sh * HC, (sh + 1) * HC)
        cols = slice(sh * HC * D, (sh + 1) * HC * D)
        WNg = WNf[:, cols]
        WEg = WEf[:, cols]
        WN_sh.append(WNg.astype(f16))
        WE_sh.append(WEg.astype(f16))
        WNT_sh.append(np.ascontiguousarray(
            WNg.T.astype(f16).reshape(HC, D, F_IN).transpose(1, 0, 2).reshape(D, HC * F_IN)))
        WET_sh.append(np.ascontiguousarray(
            WEg.T.astype(f16).reshape(HC, D, F_IN).transpose(1, 0, 2).reshape(D, HC * F_IN)))
        aN16 = np.empty((D, 3 * HC), f16)
        aE16 = np.empty((D, HC), f16)
        for k, h in enumerate(hs):
            aN16[:, 3 * k] = aNf[h, :D, 0].astype(f16)
            aN16[:, 3 * k + 1] = aNf[h, D:, 0].astype(f16)
            aN16[:, 3 * k + 2] = aEf[h, :D, 0].astype(f16)
            aE16[:, k] = aEf[h, D:, 0].astype(f16)
        aN_sh.append(aN16)
        aE_sh.append(aE16)

    in_maps = []
    for c in range(NCORES):
        si, sh = c // SH, c % SH
        sl = slice(si * RI, (si + 1) * RI)
        in_maps.append({
            "maskN": np.ascontiguousarray(maskN_T[:, sl]),
            "maskE": np.ascontiguousarray(maskE_T[:, sl]),
            "nodesT": nodesT16,
            "edgesT": edgesT16,
            "ownT": np.ascontiguousarray(nodesT16[:, sl]),
            "WNWE16": np.ascontiguousarray(
                np.concatenate([WN_sh[sh], WE_sh[sh]], axis=1)),
            "smallpk": np.ascontiguousarray(np.concatenate(
                [aN_sh[sh], aE_sh[sh], WNT_sh[sh], WET_sh[sh]], axis=1)),
        })
    return in_maps


def kernel(**inputs) -> np.ndarray:
    in_maps = _prepare_in_maps(inputs)
    nc = _get_program()
    res = run_bass_kernel_spmd(nc, in_maps, core_ids=list(range(NCORES)))
    parts = []
    for si in range(SI):
        acc = res.results[si * SH]["out"].astype(np.float32)
        for sh in range(1, SH):
            acc = acc + res.results[si * SH + sh]["out"]
        parts.append(acc)
    return np.concatenate(parts, axis=0)


# revision 12
# speedup vs baseline: 1.0075x; 1.0022x over previous
"""GAT (graph attention) kernel for Trainium2, 8 NeuronCores, v2.

Math identical to the baseline kernel: per head h,
    u[j,i] = mask[j,i] * max(P'_i * QT_j, T_j)
with P'_i = e^{0.8 src_i}, QT_j = e^{tgt_j}, T_j = e^{0.2 tgt_j}; the row
factor e^{0.2 src_i} cancels in the softmax.  Denominator comes from a
ones-column in V.

v2 changes vs baseline:
  * 2D sharding: SI i-shards x SH head-shards (SI*SH = 8 cores).  Each core
    owns RI = N/SI destination rows and HC = H/SH heads; the host adds the
    SH partial outputs per i-slice (mean over heads is linear).
  * Transposed attention matmuls: out[i,65] = ut[j,i]^T-contract V65[j,65].
    Output free size 65 instead of R -> ~2x less PE time.
  * Mask multiplies split between DVE and GpSimd (Pool) so the two engines
    work in parallel; everything else identical math.
"""

import numpy as np

from contextlib import ExitStack

import concourse.bass as bass
import concourse.bacc as bacc
import concourse.mybir as mybir
import concourse.tile as tile
from concourse.bass_utils import run_bass_kernel_spmd
from concourse.masks import make_identity

N, E, F_IN, H, D = 2048, 4096, 256, 8, 64
NCORES = 8
SI = 4                   # i-shards (destination-row shards)
SH = NCORES // SI        # head shards
RI = N // SI             # destination rows per core
IC = RI // 128           # i chunks of 128
HC = H // SH             # heads per core
NCH = N // 128           # 16 node j-chunks
ECH = E // 128           # 32 edge j-chunks
U = 2 * HC               # (head, part) broadcast rows
G = 4                    # j-chunks per fused mask-multiply group
F16 = mybir.dt.float16
F32 = mybir.dt.float32

# chunks per mask-multiply group handed to GpSimd, per group index within a
# (part, head) sweep: front-loaded so the Pool engine's backlog drains before
# the head's last (PSUM-stopping) matmuls and never runs past DVE at the tail.
GPC_N = [1, 1, 1, 1]
GPC_E = [2, 2, 2, 1, 1, 1, 1, 0]

_PROGRAM = None


def _build_program() -> bass.Bass:
    nc = bacc.Bacc("TRN2", target_bir_lowering=False, debug=False)

    maskN_d = nc.dram_tensor("maskN", [N, RI], F16, kind="ExternalInput")
    maskE_d = nc.dram_tensor("maskE", [E, RI], F16, kind="ExternalInput")
    nodesT_d = nc.dram_tensor("nodesT", [F_IN, N], F16, kind="ExternalInput")
    edgesT_d = nc.dram_tensor("edgesT", [F_IN, E], F16, kind="ExternalInput")
    ownT_d = nc.dram_tensor("ownT", [F_IN, RI], F16, kind="ExternalInput")
    # small per-head tensors packed into one DMA: [aN | aE | WNT | WET]
    SPK = 4 * HC + 2 * HC * F_IN
    spk_d = nc.dram_tensor("smallpk", [D, SPK], F16, kind="ExternalInput")
    wnwe_d = nc.dram_tensor("WNWE16", [F_IN, 2 * HC * D], F16, kind="ExternalInput")
    out_d = nc.dram_tensor("out", [RI, D], F32, kind="ExternalOutput")

    Copy = mybir.ActivationFunctionType.Copy
    Exp = mybir.ActivationFunctionType.Exp

    with tile.TileContext(nc) as tc, ExitStack() as ctx:
        singles = ctx.enter_context(tc.tile_pool(name="singles", bufs=1))
        workd = ctx.enter_context(tc.tile_pool(name="workd", bufs=20))
        small = ctx.enter_context(tc.tile_pool(name="small", bufs=6))
        psum_ht = ctx.enter_context(tc.tile_pool(name="psum_ht", bufs=3, space="PSUM"))
        psum_acc = ctx.enter_context(tc.tile_pool(name="psum_acc", bufs=3, space="PSUM"))
        psum_misc = ctx.enter_context(tc.tile_pool(name="psum_misc", bufs=2, space="PSUM"))

        # ---- persistent SBUF arrays -------------------------------------
        maskN = singles.tile([128, NCH, RI], F16, tag="maskN")
        maskE = singles.tile([128, ECH, RI], F16, tag="maskE")
        nodesT = singles.tile([128, 2, N], F16, tag="nodesT")
        edgesT = singles.tile([128, 2, E], F16, tag="edgesT")
        ownT = singles.tile([128, 2, RI], F16, tag="ownT")
        wnwe = singles.tile([128, 2, 2 * HC * D], F16, tag="wnwe")
        spk = singles.tile([D, SPK], F16, tag="spk")
        aN = spk[:, 0:3 * HC]
        aE = spk[:, 3 * HC:4 * HC]
        WNT = spk[:, 4 * HC:4 * HC + HC * F_IN].rearrange("d (h k) -> d h k", h=HC)
        WET = spk[:, 4 * HC + HC * F_IN:SPK].rearrange("d (h k) -> d h k", h=HC)
        WN = wnwe[:, :, 0:HC * D]
        WE = wnwe[:, :, HC * D:2 * HC * D]
        # per-j exponential vectors (j-chunk partition layout)
        e10n = singles.tile([128, NCH, 3 * HC], F32, tag="e10n")
        e2n = singles.tile([128, NCH, 3 * HC], F32, tag="e2n")
        e10e = singles.tile([128, ECH, HC], F32, tag="e10e")
        e2e = singles.tile([128, ECH, HC], F32, tag="e2e")
        # V tiles: [ht | 1] per (j-chunk, head); 66-wide for 4B alignment
        VT = singles.tile([128, NCH + ECH, HC, 66], F16, tag="VT")
        # P' broadcast tiles per (head, part): [j-partition-bcast, RI]
        Pb = singles.tile([128, HC, 2, RI], F16, tag="Pb")
        ptsb = singles.tile([U, RI], F16, tag="ptsb")
        selU = singles.tile([U, U * 128], F16, tag="selU")
        acc = singles.tile([128, IC, D], F32, tag="acc")

        # ---- input DMAs (critical-path tensors first) --------------------
        # issue DMAs from three queues in parallel: SP carries the score-path
        # tensors, the idle-at-start Act queue the V-path ones, and the Tensor
        # queue the masks.
        nT = nodesT_d.rearrange("(c p) n -> p c n", p=128)
        mN = maskN_d.rearrange("(t p) i -> p t i", p=128)
        mE = maskE_d.rearrange("(t p) i -> p t i", p=128)
        nc.sync.dma_start(out=spk, in_=spk_d[:, :])
        nc.sync.dma_start(out=ownT, in_=ownT_d.rearrange("(c p) n -> p c n", p=128))
        nc.sync.dma_start(out=nodesT[:, :, 0:N // 2], in_=nT[:, :, 0:N // 2])
        nc.sync.dma_start(out=maskN[:, 0:4, :], in_=mN[:, 0:4, :])
        nc.sync.dma_start(out=nodesT[:, :, N // 2:N], in_=nT[:, :, N // 2:N])
        nc.sync.dma_start(out=wnwe, in_=wnwe_d.rearrange("(c p) m -> p c m", p=128))
        nc.sync.dma_start(out=maskN[:, 4:NCH, :], in_=mN[:, 4:NCH, :])
        nc.sync.dma_start(out=edgesT, in_=edgesT_d.rearrange("(c p) n -> p c n", p=128))
        nc.sync.dma_start(out=maskE[:, 0:16, :], in_=mE[:, 0:16, :])
        nc.sync.dma_start(out=maskE[:, 16:ECH, :], in_=mE[:, 16:ECH, :])

        # preload the Exp activation table while DMAs run
        warm = singles.tile([1, 1], F32, tag="warm")
        nc.vector.memset(warm, 0.0)
        nc.scalar.activation(warm[:, :], warm[:, :], Exp)

        nc.vector.memset(acc, 0.0)
        nc.gpsimd.memset(selU, 0.0)
        nc.gpsimd.affine_select(
            out=selU, in_=selU, compare_op=mybir.AluOpType.not_equal,
            fill=1.0, base=0, pattern=[[-1, U], [0, 128]],
            channel_multiplier=1,
        )
        # ones-column scaled by H so the softmax denominator carries the 1/H
        # head-mean for free: contrib = S[:,0:64] * recip(H * denom)
        nc.vector.memset(VT[:, :, :, 64:66], 0.0)
        nc.vector.memset(VT[:, :, :, 64:65], float(H))

        # ---- attention weight vectors: wv[k, v] = sum_d W[k, hd] a[h, d] --
        wvN = singles.tile([128, 2, 3 * HC], F16, tag="wvN")
        wvE = singles.tile([128, 2, HC], F16, tag="wvE")
        pwv = psum_misc.tile([128, 2, 3 * HC], F32, tag="pm")
        pwe = psum_misc.tile([128, 2, HC], F32, tag="pm")
        for kc in range(2):
            for h in range(HC):
                nc.tensor.matmul(pwv[:, kc, 3 * h:3 * h + 3],
                                 WNT[:, h, kc * 128:(kc + 1) * 128],
                                 aN[:, 3 * h:3 * h + 3])
                nc.tensor.matmul(pwe[:, kc, h:h + 1],
                                 WET[:, h, kc * 128:(kc + 1) * 128],
                                 aE[:, h:h + 1])
        nc.vector.tensor_copy(wvN[:, :, :], pwv[:, :, :])
        nc.vector.tensor_copy(wvE[:, :, :], pwe[:, :, :])

        # ---- own-row P' = e^{0.8 src} -> broadcast tiles ------------------
        # compute the own scores already TRANSPOSED on the PE (rows u =
        # (head, part), gathered attention vectors as lhsT), then exp the
        # rows directly -- no per-chunk transposes needed.
        wvsel = small.tile([128, 2, U], F16, tag="wvsel")
        wvall = wvN[:, :, :]
        wcols = bass.AP(tensor=wvall.tensor, offset=wvall.offset,
                        ap=[wvall.ap[0], [3 * HC, 2], [3, HC], [2, 2]])
        nc.vector.tensor_copy(wvsel[:, :, :], wcols)
        pscT = psum_misc.tile([U, RI], F32, tag="pm")
        for kc in range(2):
            nc.tensor.matmul(pscT[:, :], wvsel[:, kc, :], ownT[:, kc, :],
                             start=(kc == 0), stop=(kc == 1))
        nc.scalar.activation(ptsb[:, :], pscT[:, :], Exp, scale=0.8)
        # ---- first-head P' broadcast (fast path), then scores/exps, then
        # the remaining broadcasts on the Act engine ------------------------
        def emit_pb(u):
            h, part = u // 2, u % 2
            pb = psum_misc.tile([128, RI], F32, tag="pm")
            nc.tensor.matmul(pb[:, :], selU[:, u * 128:(u + 1) * 128],
                             ptsb[:, :])
            if u < 2:
                nc.vector.tensor_copy(Pb[:, h, part, :], pb[:, :])
            else:
                nc.scalar.activation(Pb[:, h, part, :], pb[:, :], Copy)

        emit_pb(0)
        emit_pb(1)

        # ---- src/tgt scores -> per-j exponentials ------------------------
        psn = psum_misc.tile([128, NCH, 3 * HC], F32, tag="pm")
        for ch in range(NCH):
            for kc in range(2):
                nc.tensor.matmul(psn[:, ch, :],
                                 nodesT[:, kc, ch * 128:(ch + 1) * 128],
                                 wvN[:, kc, :], start=(kc == 0), stop=(kc == 1))
            if ch == 7:
                nc.scalar.activation(e10n[:, 0:8, :], psn[:, 0:8, :], Exp, scale=1.0)
                nc.scalar.activation(e2n[:, 0:8, :], psn[:, 0:8, :], Exp, scale=0.2)
        nc.scalar.activation(e10n[:, 8:NCH, :], psn[:, 8:NCH, :], Exp, scale=1.0)
        nc.scalar.activation(e2n[:, 8:NCH, :], psn[:, 8:NCH, :], Exp, scale=0.2)

        for u in range(2, U):
            emit_pb(u)

        # ---- ht = emb @ W, stored as [ht | 1] fp16 V tiles ---------------
        def emit_ht(ch):
            ph = psum_ht.tile([128, HC * D], F32, tag="ph")
            for kc in range(2):
                if ch < NCH:
                    lhsT = nodesT[:, kc, ch * 128:(ch + 1) * 128]
                    rhs = WN[:, kc, :]
                else:
                    lhsT = edgesT[:, kc, (ch - NCH) * 128:(ch - NCH + 1) * 128]
                    rhs = WE[:, kc, :]
                nc.tensor.matmul(ph[:, :], lhsT, rhs, start=(kc == 0), stop=(kc == 1))
            nc.scalar.activation(
                VT[:, ch, :, 0:64],
                ph[:, :].rearrange("p (h d) -> p h d", h=HC),
                Copy,
            )

        for ch in range(NCH):
            emit_ht(ch)

        def emit_edges_prep():
            pse = psum_misc.tile([128, ECH, HC], F32, tag="pm")
            for ch in range(ECH):
                for kc in range(2):
                    nc.tensor.matmul(pse[:, ch, :],
                                     edgesT[:, kc, ch * 128:(ch + 1) * 128],
                                     wvE[:, kc, :], start=(kc == 0), stop=(kc == 1))
            nc.scalar.activation(e10e[:, :, :], pse[:, :, :], Exp, scale=1.0)
            nc.scalar.activation(e2e[:, :, :], pse[:, :, :], Exp, scale=0.2)
            for ch in range(NCH, NCH + ECH):
                emit_ht(ch)

        # ---- main loop ---------------------------------------------------
        # u = mask * max(P'*QT, T); S^T[i, 0:65] += ut-chunk^T contract V65.
        # The mask multiply is split per group: first G-GPC chunks on DVE,
        # last GPC chunks on GpSimd, so both engines fill in parallel and the
        # in-order PE sees the DVE-made chunks first.
        for part in range(2):
            njt = NCH if part == 0 else ECH
            for h in range(HC):
                if part == 0 and h == 1:
                    emit_edges_prep()
                Sp = psum_acc.tile([128, IC, 65], F32, tag="Sacc")
                for gi, jt0 in enumerate(range(0, njt, G)):
                    dt_ = workd.tile([128, G, RI], F16, tag="Dt")
                    gpts = (part == 0 and gi % 2 == 0) or (part == 1 and gi == 0)
                    for g in range(G):
                        jt = jt0 + g
                        if part == 0:
                            q10 = e10n[:, jt, 3 * h + 1:3 * h + 2]
                            q2 = e2n[:, jt, 3 * h + 1:3 * h + 2]
                        else:
                            q10 = e10e[:, jt, h:h + 1]
                            q2 = e2e[:, jt, h:h + 1]
                        # for alternating N-groups the gp-owned chunk's score
                        # op also runs on GpSimd, keeping its chain on-engine
                        eng = nc.gpsimd if (gpts and g == G - 1) else nc.vector
                        eng.tensor_scalar(
                            out=dt_[:, g, :], in0=Pb[:, h, part, :],
                            scalar1=q10, scalar2=q2,
                            op0=mybir.AluOpType.mult, op1=mybir.AluOpType.max,
                        )
                    ut = dt_
                    msk = maskN if part == 0 else maskE
                    nd = G - (GPC_N if part == 0 else GPC_E)[gi]
                    if nd > 0:
                        nc.vector.tensor_mul(ut[:, 0:nd, :], dt_[:, 0:nd, :],
                                             msk[:, jt0:jt0 + nd, :])
                    if nd < G:
                        nc.gpsimd.tensor_mul(ut[:, nd:G, :], dt_[:, nd:G, :],
                                             msk[:, jt0 + nd:jt0 + G, :])
                    for g in range(G):
                        jt = jt0 + g
                        vch = jt if part == 0 else NCH + jt
                        # one start marks Sp's whole PSUM zero-region pending:
                        # each ic slice's first write then overwrites, later
                        # writes accumulate (per-ic starts would re-mark and
                        # wipe sibling slices' first chunk).
                        for ic in range(IC):
                            nc.tensor.matmul(
                                Sp[:, ic, :],
                                ut[:, g, ic * 128:(ic + 1) * 128],
                                VT[:, vch, h, 0:65],
                                start=(jt == 0 and ic == 0),
                                stop=(jt == njt - 1 and ic == IC - 1),
                            )
                # ---- normalize + accumulate ------------------------------
                contrib = small.tile([128, IC, D], F32, tag="contrib")
                rec = small.tile([128, IC, 1], F32, tag="rec")
                nc.vector.reciprocal(rec[:, :, :], Sp[:, :, 64:65])
                last = part == 1 and h == HC - 1
                if last:
                    # all-DVE tail, one batched op: rec broadcast along d via
                    # a stride-0 AP so all IC chunks normalize at once
                    rsl = rec[:, :, :]
                    rb = bass.AP(tensor=rsl.tensor, offset=rsl.offset,
                                 ap=[rsl.ap[0], [1, IC], [0, D]])
                    nc.vector.tensor_mul(contrib[:, :, :], Sp[:, :, 0:64], rb)
                else:
                    for ic in range(IC):
                        nc.scalar.activation(contrib[:, ic, :],
                                             Sp[:, ic, 0:64], Copy,
                                             scale=rec[:, ic, :])
                if last:
                    oD = out_d.rearrange("(c p) d -> p c d", p=128)
                    for half in range(2):
                        i0, i1 = half * (IC // 2), (half + 1) * (IC // 2)
                        nc.vector.tensor_add(acc[:, i0:i1, :], acc[:, i0:i1, :],
                                             contrib[:, i0:i1, :])
                        nc.sync.dma_start(out=oD[:, i0:i1, :],
                                          in_=acc[:, i0:i1, :])
                else:
                    nc.gpsimd.tensor_add(acc[:, :, :], acc[:, :, :],
                                         contrib[:, :, :])

    return nc


def _get_program() -> bass.Bass:
    global _PROGRAM
    if _PROGRAM is None:
        nc = _build_program()
        nc.finalize()
        _PROGRAM = nc
    return _PROGRAM


def _prepare_in_maps(inputs) -> list:
    nodes = np.ascontiguousarray(np.asarray(inputs["nodes_embeddings"], np.float32))
    edges = np.ascontiguousarray(np.asarray(inputs["edges_embeddings"], np.float32))
    WNf = np.asarray(inputs["WN"], np.float32)
    WEf = np.asarray(inputs["WE"], np.float32)
    aNf = np.asarray(inputs["aN"], np.float32)
    aEf = np.asarray(inputs["aE"], np.float32)
    mat_nodes = np.asarray(inputs["mat_nodes"])
    mat_edges = np.asarray(inputs["mat_edges"])

    f16 = np.float16
    nodesT16 = np.ascontiguousarray(nodes.T.astype(f16))
    edgesT16 = np.ascontiguousarray(edges.T.astype(f16))
    maskN_T = np.ascontiguousarray(mat_nodes.astype(f16).T)  # [j, i_global]
    maskE_T = np.ascontiguousarray(mat_edges.astype(f16).T)

    # per head-shard weight slices
    WN_sh, WE_sh, WNT_sh, WET_sh, aN_sh, aE_sh = [], [], [], [], [], []
    for sh in range(SH):
        hs = range(sh * HC, (sh + 1) * HC)
        cols = slice(sh * HC * D, (sh + 1) * HC * D)
        WNg = WNf[:, cols]
        WEg = WEf[:, cols]
        WN_sh.append(WNg.astype(f16))
        WE_sh.append(WEg.astype(f16))
        WNT_sh.append(np.ascontiguousarray(
            WNg.T.astype(f16).reshape(HC, D, F_IN).transpose(1, 0, 2).reshape(D, HC * F_IN)))
        WET_sh.append(np.ascontiguousarray(
            WEg.T.astype(f16).reshape(HC, D, F_IN).transpose(1, 0, 2).reshape(D, HC * F_IN)))
        aN16 = np.empty((D, 3 * HC), f16)
        aE16 = np.empty((D, HC), f16)
        for k, h in enumerate(hs):
            aN16[:, 3 * k] = aNf[h, :D, 0].astype(f16)
            aN16[:, 3 * k + 1] = aNf[h, D:, 0].astype(f16)
            aN16[:, 3 * k + 2] = aEf[h, :D, 0].astype(f16)
            aE16[:, k] = aEf[h, D:, 0].astype(f16)
        aN_sh.append(aN16)
        aE_sh.append(aE16)

    in_maps = []
    for c in range(NCORES):
        si, sh = c // SH, c % SH
        sl = slice(si * RI, (si + 1) * RI)
        in_maps.append({
            "maskN": np.ascontiguousarray(maskN_T[:, sl]),
            "maskE": np.ascontiguousarray(maskE_T[:, sl]),
            "nodesT": nodesT16,
            "edgesT": edgesT16,
            "ownT": np.ascontiguousarray(nodesT16[:, sl]),
            "WNWE16": np.ascontiguousarray(
                np.concatenate([WN_sh[sh], WE_sh[sh]], axis=1)),
            "smallpk": np.ascontiguousarray(np.concatenate(
                [aN_sh[sh], aE_sh[sh], WNT_sh[sh], WET_sh[sh]], axis=1)),
        })
    return in_maps


def kernel(**inputs) -> np.ndarray:
    in_maps = _prepare_in_maps(inputs)
    nc = _get_program()
    res = run_bass_kernel_spmd(nc, in_maps, core_ids=list(range(NCORES)))
    parts = []
    for si in range(SI):
        acc = res.results[si * SH]["out"].astype(np.float32)
        for sh in range(1, SH):
            acc = acc + res.results[si * SH + sh]["out"]
        parts.append(acc)
    return np.concatenate(parts, axis=0)
